# revision 38
# baseline (speedup 1.0000x reference)
"""Trainium2 Bass kernel for nn_DiffusionLayer (ADI diffusion, 10 steps).

Mathematical collapse: every sweep of the ADI scheme is a fixed tridiagonal
solve shared by all rows (the coefficients depend only on the size-128
parameter vectors and the time index, never on u). Each x-sweep is a right
multiplication V <- V @ Mx^T and each y-sweep a left multiplication
V <- My @ V of the 128x128 image V. Left and right multiplications commute,
so the whole 30-sweep scheme is

    V_out = L @ V @ R,   L = My_10 ... My_1,   R = Mx_1^T Mx_2^T ... Mx_20^T

with L, R computed on host in float64 (including the reference's EPS
perturbations of the Thomas recurrences).

Device pipeline (fp16 wire, fp32 PSUM accumulation). The host owns both
DRAM layouts, so all transposes are free numpy work:

  host:   upload u as [w, img, h]  (so each image arrives as V^T)
  mmA:    per image   out = (V^T)^T R = V R     [h part, w' free]
          (lhsT = V^T slice, stationary reload per image)
  copy2:  DVE   PSUM fp32 -> SBUF fp16 quad [h, (4img, w')]
  mmB:    batched, shared stationary L^T, moving = quad [h, 4*128]
          out = L (V R) = final                 [h', (4img, w')]
  copy3:  Act   PSUM fp32 -> SBUF fp16 [h', (img, w')]
  out:    DMA to DRAM [h', img, w']; host moves axis back to [img, h', w']

Both DMA directions are per-partition contiguous (4KB descriptors).
Sharding: pure data parallelism, 2048 images -> 256 per core across 8 cores.
"""

import numpy as np

import concourse.mybir as mybir
import concourse.tile as tile
from concourse import bacc
from concourse.bass_utils import run_bass_kernel_spmd

N_CORES = 8
BATCH = 2048
S = 128
PER_CORE = BATCH // N_CORES  # 256

SIZE, DT, DX, DY, NUM_STEPS, EPS = 128, 0.01, 1.0, 1.0, 10, 1e-6

GIMG = 16                 # images per input DMA group
NGRP = PER_CORE // GIMG   # 16
QUAD = 4                  # images per PSUM bank / mmB moving batch
OCH = 16                  # images per output DMA chunk


# ----------------------------------------------------------------- host math
def _smooth3(v):
    vp = np.pad(v, (1, 1), mode="edge")
    return (vp[:-2] + vp[1:-1] + vp[2:]) / 3.0


def _thomas_matrix(a, b, c):
    """Matrix M of the reference thomas() linear map d -> x (includes EPS)."""
    n = len(b)
    dn = np.empty(n)
    cs = np.empty(n)
    dn[0] = b[0] + EPS
    cs[0] = c[0] / dn[0]
    for i in range(1, n):
        dn[i] = b[i] - a[i] * cs[i - 1] + EPS
        cs[i] = c[i] / dn[i]
    ds = np.empty((n, n))
    ds[0] = np.eye(n)[0] / dn[0]
    eye = np.eye(n)
    for i in range(1, n):
        ds[i] = (eye[i] - a[i] * ds[i - 1]) / dn[i]
    x = np.empty((n, n))
    x[n - 1] = ds[n - 1]
    for i in range(n - 2, -1, -1):
        x[i] = ds[i] - cs[i] * x[i + 1]
    return x


def _sweep_matrix(vec, dt, dh):
    coeff = _smooth3(vec) * dt / dh**2
    a = -coeff
    c = -coeff
    b = 1.0 + 2.0 * coeff
    b = b.copy()
    b[0] = 1.0 + coeff[0]
    b[-1] = 1.0 + coeff[-1]
    return _thomas_matrix(a, b, c)


def _coef(base, lin, quad, t):
    return np.clip(base + lin * t + quad * t * t, EPS, None)


def _build_lr(abx, atx, aqx, bby, bty, bqy):
    """L (y-operator product) and R (x-operator product) in float64."""
    L = np.eye(SIZE)
    R = np.eye(SIZE)
    t = 0.0
    for _ in range(NUM_STEPS):
        Mx = _sweep_matrix(_coef(abx, atx, aqx, t), DT / 2, DX)
        R = R @ Mx.T
        t += DT / 2
        My = _sweep_matrix(_coef(bby, bty, bqy, t), DT, DY)
        L = My @ L
        t += DT / 2
        Mx = _sweep_matrix(_coef(abx, atx, aqx, t), DT / 2, DX)
        R = R @ Mx.T
    return L, R


# ------------------------------------------------------------- device kernel
_NC_CACHE = {}


def _build_nc():
    if "nc" in _NC_CACHE:
        return _NC_CACHE["nc"]
    f16 = mybir.dt.float16
    f32 = mybir.dt.float32
    nc = bacc.Bacc(None)
    # input laid out [w, img, h]; output [h', img, w'] (host fixes both up)
    u_in = nc.dram_tensor("u", [S, PER_CORE, S], f16, kind="ExternalInput")
    lt_in = nc.dram_tensor("lt", [S, S], f16, kind="ExternalInput")
    r_in = nc.dram_tensor("rm", [S, S], f16, kind="ExternalInput")
    u_out = nc.dram_tensor("out", [S, PER_CORE, S], f16, kind="ExternalOutput")

    with tile.TileContext(nc) as tc:
        with (
            tc.tile_pool(name="mats", bufs=1) as mats,
            tc.tile_pool(name="vt", bufs=16) as vtp,
            tc.tile_pool(name="vrq", bufs=8) as vrp,
            tc.tile_pool(name="outc", bufs=16) as outp,
            tc.tile_pool(name="psA", bufs=5, space="PSUM") as psA,
            tc.tile_pool(name="psB", bufs=3, space="PSUM") as psB,
        ):
            lt_s = mats.tile([S, S], f16)   # L^T
            r_s = mats.tile([S, S], f16)    # R

            # vt pool holds every group, so input DMA issues never block on
            # buffer recycling: Sync finishes issuing ALL inputs early and
            # the output-chunk issues behind them enter the DMA rings as
            # soon as their data is ready. The first groups arrive in
            # 4-image pieces so the PE starts early; weights ride Act's
            # HWDGE queue in parallel.
            vt_tiles = []
            for g in range(NGRP):
                vt = vtp.tile([S, GIMG, S], f16)  # [w, img, h]
                g0 = g * GIMG
                if g < 2:
                    for c in range(0, GIMG, QUAD):
                        nc.sync.dma_start(
                            out=vt[:, c : c + QUAD, :],
                            in_=u_in[:, g0 + c : g0 + c + QUAD, :],
                        )
                        if g == 0 and c == 0:
                            nc.scalar.dma_start(out=r_s[:], in_=r_in[:])
                            nc.scalar.dma_start(out=lt_s[:], in_=lt_in[:])
                else:
                    nc.sync.dma_start(out=vt[:], in_=u_in[:, g0 : g0 + GIMG, :])
                vt_tiles.append(vt)

            # dedicated engines for the two PSUM->SBUF copies
            # (GPSIMD cannot access PSUM on TRN2)
            dve = lambda o, i: nc.vector.tensor_copy(o, i)
            act = lambda o, i: nc.scalar.copy(o, i)
            cp2_eng = [dve, dve]
            cp3_eng = [act, act]

            for c in range(PER_CORE // OCH):
                oc = outp.tile([S, OCH, S], f16)  # [h', img, w']
                for q in range(OCH // QUAD):
                    i0 = c * OCH + q * QUAD  # global image index
                    qg = i0 // QUAD          # global quad index
                    vt = vt_tiles[i0 // GIMG]
                    pa = psA.tile([S, QUAD, S], f32)
                    for j in range(QUAD):
                        nc.tensor.matmul(
                            pa[:, j, :],
                            vt[:, i0 % GIMG + j, :],
                            r_s[:],
                        )
                    vq = vrp.tile([S, QUAD, S], f16)  # [h, (4img, w')]
                    cp2_eng[qg % 2](vq[:], pa[:])
                    pb = psB.tile([S, QUAD, S], f32)  # [h', (4img, w')]
                    nc.tensor.matmul(pb[:], lt_s[:], vq[:])
                    cp3_eng[qg % 2](oc[:, q * QUAD : (q + 1) * QUAD, :], pb[:])
                o0 = c * OCH
                if c == PER_CORE // OCH - 1:
                    # split the final chunk so the tail DMA is short
                    h = OCH // 2
                    nc.sync.dma_start(
                        out=u_out[:, o0 : o0 + h, :], in_=oc[:, :h, :]
                    )
                    nc.sync.dma_start(
                        out=u_out[:, o0 + h : o0 + OCH, :], in_=oc[:, h:, :]
                    )
                else:
                    nc.sync.dma_start(
                        out=u_out[:, o0 : o0 + OCH, :], in_=oc[:]
                    )

    nc.finalize()
    _NC_CACHE["nc"] = nc
    return nc


def _prep_in_maps(inputs):
    """Host-side prep shared by kernel() and the profiling harness."""
    u = np.asarray(inputs["u"], dtype=np.float32)
    assert u.shape == (BATCH, 1, S, S)
    L, R = _build_lr(
        np.asarray(inputs["alpha_base_x"], dtype=np.float64),
        np.asarray(inputs["alpha_time_coeff_x"], dtype=np.float64),
        np.asarray(inputs["alpha_time_quad_x"], dtype=np.float64),
        np.asarray(inputs["beta_base_y"], dtype=np.float64),
        np.asarray(inputs["beta_time_coeff_y"], dtype=np.float64),
        np.asarray(inputs["beta_time_quad_y"], dtype=np.float64),
    )
    lt16 = np.ascontiguousarray(L.T.astype(np.float16))
    r16 = np.ascontiguousarray(R.astype(np.float16))
    u16 = u[:, 0].astype(np.float16)  # (BATCH, S, S) = [img, h, w]
    return [
        {
            # [w, img, h]: each image uploaded pre-transposed
            "u": np.ascontiguousarray(
                u16[c * PER_CORE : (c + 1) * PER_CORE].transpose(2, 0, 1)
            ),
            "lt": lt16,
            "rm": r16,
        }
        for c in range(N_CORES)
    ]


# ---------------------------------------------------------------- entrypoint
def kernel(**inputs) -> np.ndarray:
    in_maps = _prep_in_maps(inputs)
    nc = _build_nc()
    res = run_bass_kernel_spmd(nc, in_maps, list(range(N_CORES)))
    # device emits [h', img, w'] per core; reassemble to [img, h', w']
    out = np.concatenate([r["out"].transpose(1, 0, 2) for r in res.results], axis=0)
    return np.ascontiguousarray(out).reshape(BATCH, 1, S, S).astype(np.float32)


if __name__ == "__main__":
    rng = np.random.default_rng(0)
    fake = {
        "u": rng.standard_normal((BATCH, 1, S, S), dtype=np.float32),
        "alpha_base_x": np.full(S, 2.0, np.float32),
        "alpha_base_y": np.full(S, 2.0, np.float32),
        "beta_base_x": np.full(S, 2.0, np.float32),
        "beta_base_y": np.full(S, 2.0, np.float32),
        "alpha_time_coeff_x": 0.01 * rng.standard_normal(S).astype(np.float32),
        "alpha_time_coeff_y": 0.01 * rng.standard_normal(S).astype(np.float32),
        "beta_time_coeff_x": 0.01 * rng.standard_normal(S).astype(np.float32),
        "beta_time_coeff_y": 0.01 * rng.standard_normal(S).astype(np.float32),
        "alpha_time_quad_x": 0.01 * rng.standard_normal(S).astype(np.float32),
        "alpha_time_quad_y": 0.01 * rng.standard_normal(S).astype(np.float32),
        "beta_time_quad_x": 0.01 * rng.standard_normal(S).astype(np.float32),
        "beta_time_quad_y": 0.01 * rng.standard_normal(S).astype(np.float32),
    }
    out = kernel(**fake)
    print("kernel output:", out.shape, out.dtype)


# revision 41
# speedup vs baseline: 1.1328x; 1.1328x over previous
"""Trainium2 Bass kernel for nn_DiffusionLayer (ADI diffusion, 10 steps).

Mathematical collapse: every sweep of the ADI scheme is a fixed tridiagonal
solve shared by all rows (the coefficients depend only on the size-128
parameter vectors and the time index, never on u). Each x-sweep is a right
multiplication V <- V @ Mx^T and each y-sweep a left multiplication
V <- My @ V of the 128x128 image V. Left and right multiplications commute,
so the whole 30-sweep scheme is

    V_out = L @ V @ R,   L = My_10 ... My_1,   R = Mx_1^T Mx_2^T ... Mx_20^T

with L, R computed on host in float64 (including the reference's EPS
perturbations of the Thomas recurrences).

Device pipeline (fp16 wire, fp32 PSUM accumulation). The host owns both
DRAM layouts, so all transposes are free numpy work:

  host:   upload u as [w, img, h]  (so each image arrives as V^T)
  mmA:    per image   out = (V^T)^T R = V R     [h part, w' free]
          (lhsT = V^T slice, stationary reload per image)
  copy2:  DVE   PSUM fp32 -> SBUF fp16 quad [h, (4img, w')]
  mmB:    batched, shared stationary L^T, moving = quad [h, 4*128]
          out = L (V R) = final                 [h', (4img, w')]
  copy3:  Act   PSUM fp32 -> SBUF fp16 [h', (img, w')]
  out:    DMA to DRAM [h', img, w']; host moves axis back to [img, h', w']

Both DMA directions are per-partition contiguous (4KB descriptors).
Sharding: pure data parallelism, 2048 images -> 256 per core across 8 cores.
"""

import numpy as np

import concourse.mybir as mybir
import concourse.tile as tile
from concourse import bacc
from concourse.bass_utils import run_bass_kernel_spmd

N_CORES = 8
BATCH = 2048
S = 128
PER_CORE = BATCH // N_CORES  # 256

SIZE, DT, DX, DY, NUM_STEPS, EPS = 128, 0.01, 1.0, 1.0, 10, 1e-6

GIMG = 16                 # images per input DMA group
NGRP = PER_CORE // GIMG   # 16
QUAD = 4                  # images per PSUM bank / mmB moving batch
OCH = 16                  # images per output DMA chunk


# ----------------------------------------------------------------- host math
def _smooth3(v):
    vp = np.pad(v, (1, 1), mode="edge")
    return (vp[:-2] + vp[1:-1] + vp[2:]) / 3.0


def _thomas_matrix(a, b, c):
    """Matrix M of the reference thomas() linear map d -> x (includes EPS)."""
    n = len(b)
    dn = np.empty(n)
    cs = np.empty(n)
    dn[0] = b[0] + EPS
    cs[0] = c[0] / dn[0]
    for i in range(1, n):
        dn[i] = b[i] - a[i] * cs[i - 1] + EPS
        cs[i] = c[i] / dn[i]
    ds = np.empty((n, n))
    ds[0] = np.eye(n)[0] / dn[0]
    eye = np.eye(n)
    for i in range(1, n):
        ds[i] = (eye[i] - a[i] * ds[i - 1]) / dn[i]
    x = np.empty((n, n))
    x[n - 1] = ds[n - 1]
    for i in range(n - 2, -1, -1):
        x[i] = ds[i] - cs[i] * x[i + 1]
    return x


def _sweep_matrix(vec, dt, dh):
    coeff = _smooth3(vec) * dt / dh**2
    a = -coeff
    c = -coeff
    b = 1.0 + 2.0 * coeff
    b = b.copy()
    b[0] = 1.0 + coeff[0]
    b[-1] = 1.0 + coeff[-1]
    return _thomas_matrix(a, b, c)


def _coef(base, lin, quad, t):
    return np.clip(base + lin * t + quad * t * t, EPS, None)


def _build_lr(abx, atx, aqx, bby, bty, bqy):
    """L (y-operator product) and R (x-operator product) in float64."""
    L = np.eye(SIZE)
    R = np.eye(SIZE)
    t = 0.0
    for _ in range(NUM_STEPS):
        Mx = _sweep_matrix(_coef(abx, atx, aqx, t), DT / 2, DX)
        R = R @ Mx.T
        t += DT / 2
        My = _sweep_matrix(_coef(bby, bty, bqy, t), DT, DY)
        L = My @ L
        t += DT / 2
        Mx = _sweep_matrix(_coef(abx, atx, aqx, t), DT / 2, DX)
        R = R @ Mx.T
    return L, R


# ------------------------------------------------------------- device kernel
_NC_CACHE = {}


def _build_nc():
    if "nc" in _NC_CACHE:
        return _NC_CACHE["nc"]
    f16 = mybir.dt.float16
    f32 = mybir.dt.float32
    nc = bacc.Bacc(None)
    # input laid out [w, img, h]; output [h', img, w'] (host fixes both up)
    u_in = nc.dram_tensor("u", [S, PER_CORE, S], f16, kind="ExternalInput")
    lt_in = nc.dram_tensor("lt", [S, S], f16, kind="ExternalInput")
    r_in = nc.dram_tensor("rm", [S, S], f16, kind="ExternalInput")
    u_out = nc.dram_tensor("out", [S, PER_CORE, S], f16, kind="ExternalOutput")

    with tile.TileContext(nc) as tc:
        with (
            tc.tile_pool(name="mats", bufs=1) as mats,
            tc.tile_pool(name="vt", bufs=16) as vtp,
            tc.tile_pool(name="vrq", bufs=8) as vrp,
            tc.tile_pool(name="outc", bufs=16) as outp,
            tc.tile_pool(name="psA", bufs=5, space="PSUM") as psA,
            tc.tile_pool(name="psB", bufs=3, space="PSUM") as psB,
        ):
            lt_s = mats.tile([S, S], f16)   # L^T
            r_s = mats.tile([S, S], f16)    # R

            # vt pool holds every group, so input DMA issues never block on
            # buffer recycling: Sync finishes issuing ALL inputs early and
            # the output-chunk issues behind them enter the DMA rings as
            # soon as their data is ready. The first groups arrive in
            # 4-image pieces so the PE starts early; weights ride Act's
            # HWDGE queue in parallel.
            vt_tiles = []
            for g in range(NGRP):
                vt = vtp.tile([S, GIMG, S], f16)  # [w, img, h]
                g0 = g * GIMG
                if g < 2:
                    for c in range(0, GIMG, QUAD):
                        nc.sync.dma_start(
                            out=vt[:, c : c + QUAD, :],
                            in_=u_in[:, g0 + c : g0 + c + QUAD, :],
                        )
                        if g == 0 and c == 0:
                            nc.scalar.dma_start(out=r_s[:], in_=r_in[:])
                            nc.scalar.dma_start(out=lt_s[:], in_=lt_in[:])
                else:
                    nc.sync.dma_start(out=vt[:], in_=u_in[:, g0 : g0 + GIMG, :])
                vt_tiles.append(vt)

            # dedicated engines for the two PSUM->SBUF copies
            # (GPSIMD cannot access PSUM on TRN2). mmB is software-pipelined
            # one quad behind mmA: the PE is in-order, so emitting mmB(q)
            # right after copy2(q) would head-of-line-block the next quad's
            # mmA matmuls while copy2 is still on the DVE. With the one-quad
            # lag, copy2 has a full quad of slack before the PE needs it.
            NCH = PER_CORE // OCH
            QPC = OCH // QUAD
            oc_tiles = {}

            def flush_mmB(p):
                vq, c, q = p
                oc = oc_tiles[c]
                pb = psB.tile([S, QUAD, S], f32)  # [h', (4img, w')]
                nc.tensor.matmul(pb[:], lt_s[:], vq[:])
                nc.scalar.copy(oc[:, q * QUAD : (q + 1) * QUAD, :], pb[:])
                if q == QPC - 1:
                    o0 = c * OCH
                    if c == NCH - 1:
                        # split the final chunk so the tail DMA is short
                        h = OCH // 2
                        nc.sync.dma_start(
                            out=u_out[:, o0 : o0 + h, :], in_=oc[:, :h, :]
                        )
                        nc.sync.dma_start(
                            out=u_out[:, o0 + h : o0 + OCH, :], in_=oc[:, h:, :]
                        )
                    else:
                        nc.sync.dma_start(
                            out=u_out[:, o0 : o0 + OCH, :], in_=oc[:]
                        )

            pending = None
            for c in range(NCH):
                oc = outp.tile([S, OCH, S], f16, name="oc")  # [h', img, w']
                oc_tiles[c] = oc
                for q in range(QPC):
                    i0 = c * OCH + q * QUAD  # global image index
                    vt = vt_tiles[i0 // GIMG]
                    pa = psA.tile([S, QUAD, S], f32)
                    for j in range(QUAD):
                        nc.tensor.matmul(
                            pa[:, j, :],
                            vt[:, i0 % GIMG + j, :],
                            r_s[:],
                        )
                    vq = vrp.tile([S, QUAD, S], f16)  # [h, (4img, w')]
                    nc.vector.tensor_copy(vq[:], pa[:])
                    if pending is not None:
                        flush_mmB(pending)
                    pending = (vq, c, q)
            flush_mmB(pending)

    nc.finalize()
    _NC_CACHE["nc"] = nc
    return nc


def _prep_in_maps(inputs):
    """Host-side prep shared by kernel() and the profiling harness."""
    u = np.asarray(inputs["u"], dtype=np.float32)
    assert u.shape == (BATCH, 1, S, S)
    L, R = _build_lr(
        np.asarray(inputs["alpha_base_x"], dtype=np.float64),
        np.asarray(inputs["alpha_time_coeff_x"], dtype=np.float64),
        np.asarray(inputs["alpha_time_quad_x"], dtype=np.float64),
        np.asarray(inputs["beta_base_y"], dtype=np.float64),
        np.asarray(inputs["beta_time_coeff_y"], dtype=np.float64),
        np.asarray(inputs["beta_time_quad_y"], dtype=np.float64),
    )
    lt16 = np.ascontiguousarray(L.T.astype(np.float16))
    r16 = np.ascontiguousarray(R.astype(np.float16))
    u16 = u[:, 0].astype(np.float16)  # (BATCH, S, S) = [img, h, w]
    return [
        {
            # [w, img, h]: each image uploaded pre-transposed
            "u": np.ascontiguousarray(
                u16[c * PER_CORE : (c + 1) * PER_CORE].transpose(2, 0, 1)
            ),
            "lt": lt16,
            "rm": r16,
        }
        for c in range(N_CORES)
    ]


# ---------------------------------------------------------------- entrypoint
def kernel(**inputs) -> np.ndarray:
    in_maps = _prep_in_maps(inputs)
    nc = _build_nc()
    res = run_bass_kernel_spmd(nc, in_maps, list(range(N_CORES)))
    # device emits [h', img, w'] per core; reassemble to [img, h', w']
    out = np.concatenate([r["out"].transpose(1, 0, 2) for r in res.results], axis=0)
    return np.ascontiguousarray(out).reshape(BATCH, 1, S, S).astype(np.float32)


if __name__ == "__main__":
    rng = np.random.default_rng(0)
    fake = {
        "u": rng.standard_normal((BATCH, 1, S, S), dtype=np.float32),
        "alpha_base_x": np.full(S, 2.0, np.float32),
        "alpha_base_y": np.full(S, 2.0, np.float32),
        "beta_base_x": np.full(S, 2.0, np.float32),
        "beta_base_y": np.full(S, 2.0, np.float32),
        "alpha_time_coeff_x": 0.01 * rng.standard_normal(S).astype(np.float32),
        "alpha_time_coeff_y": 0.01 * rng.standard_normal(S).astype(np.float32),
        "beta_time_coeff_x": 0.01 * rng.standard_normal(S).astype(np.float32),
        "beta_time_coeff_y": 0.01 * rng.standard_normal(S).astype(np.float32),
        "alpha_time_quad_x": 0.01 * rng.standard_normal(S).astype(np.float32),
        "alpha_time_quad_y": 0.01 * rng.standard_normal(S).astype(np.float32),
        "beta_time_quad_x": 0.01 * rng.standard_normal(S).astype(np.float32),
        "beta_time_quad_y": 0.01 * rng.standard_normal(S).astype(np.float32),
    }
    out = kernel(**fake)
    print("kernel output:", out.shape, out.dtype)


# revision 42
# speedup vs baseline: 1.1386x; 1.0051x over previous
"""Trainium2 Bass kernel for nn_DiffusionLayer (ADI diffusion, 10 steps).

Mathematical collapse: every sweep of the ADI scheme is a fixed tridiagonal
solve shared by all rows (the coefficients depend only on the size-128
parameter vectors and the time index, never on u). Each x-sweep is a right
multiplication V <- V @ Mx^T and each y-sweep a left multiplication
V <- My @ V of the 128x128 image V. Left and right multiplications commute,
so the whole 30-sweep scheme is

    V_out = L @ V @ R,   L = My_10 ... My_1,   R = Mx_1^T Mx_2^T ... Mx_20^T

with L, R computed on host in float64 (including the reference's EPS
perturbations of the Thomas recurrences).

Device pipeline (fp16 wire, fp32 PSUM accumulation). The host owns both
DRAM layouts, so all transposes are free numpy work:

  host:   upload u as [w, img, h]  (so each image arrives as V^T)
  mmA:    per image   out = (V^T)^T R = V R     [h part, w' free]
          (lhsT = V^T slice, stationary reload per image)
  copy2:  DVE   PSUM fp32 -> SBUF fp16 quad [h, (4img, w')]
  mmB:    batched, shared stationary L^T, moving = quad [h, 4*128]
          out = L (V R) = final                 [h', (4img, w')]
  copy3:  Act   PSUM fp32 -> SBUF fp16 [h', (img, w')]
  out:    DMA to DRAM [h', img, w']; host moves axis back to [img, h', w']

Both DMA directions are per-partition contiguous (4KB descriptors).
Sharding: pure data parallelism, 2048 images -> 256 per core across 8 cores.
"""

import numpy as np

import concourse.mybir as mybir
import concourse.tile as tile
from concourse import bacc
from concourse.bass_utils import run_bass_kernel_spmd

N_CORES = 8
BATCH = 2048
S = 128
PER_CORE = BATCH // N_CORES  # 256

SIZE, DT, DX, DY, NUM_STEPS, EPS = 128, 0.01, 1.0, 1.0, 10, 1e-6

GIMG = 16                 # images per input DMA group
NGRP = PER_CORE // GIMG   # 16
QUAD = 4                  # images per PSUM bank / mmB moving batch
OCH = 16                  # images per output DMA chunk


# ----------------------------------------------------------------- host math
def _smooth3(v):
    vp = np.pad(v, (1, 1), mode="edge")
    return (vp[:-2] + vp[1:-1] + vp[2:]) / 3.0


def _thomas_matrix(a, b, c):
    """Matrix M of the reference thomas() linear map d -> x (includes EPS)."""
    n = len(b)
    dn = np.empty(n)
    cs = np.empty(n)
    dn[0] = b[0] + EPS
    cs[0] = c[0] / dn[0]
    for i in range(1, n):
        dn[i] = b[i] - a[i] * cs[i - 1] + EPS
        cs[i] = c[i] / dn[i]
    ds = np.empty((n, n))
    ds[0] = np.eye(n)[0] / dn[0]
    eye = np.eye(n)
    for i in range(1, n):
        ds[i] = (eye[i] - a[i] * ds[i - 1]) / dn[i]
    x = np.empty((n, n))
    x[n - 1] = ds[n - 1]
    for i in range(n - 2, -1, -1):
        x[i] = ds[i] - cs[i] * x[i + 1]
    return x


def _sweep_matrix(vec, dt, dh):
    coeff = _smooth3(vec) * dt / dh**2
    a = -coeff
    c = -coeff
    b = 1.0 + 2.0 * coeff
    b = b.copy()
    b[0] = 1.0 + coeff[0]
    b[-1] = 1.0 + coeff[-1]
    return _thomas_matrix(a, b, c)


def _coef(base, lin, quad, t):
    return np.clip(base + lin * t + quad * t * t, EPS, None)


def _build_lr(abx, atx, aqx, bby, bty, bqy):
    """L (y-operator product) and R (x-operator product) in float64."""
    L = np.eye(SIZE)
    R = np.eye(SIZE)
    t = 0.0
    for _ in range(NUM_STEPS):
        Mx = _sweep_matrix(_coef(abx, atx, aqx, t), DT / 2, DX)
        R = R @ Mx.T
        t += DT / 2
        My = _sweep_matrix(_coef(bby, bty, bqy, t), DT, DY)
        L = My @ L
        t += DT / 2
        Mx = _sweep_matrix(_coef(abx, atx, aqx, t), DT / 2, DX)
        R = R @ Mx.T
    return L, R


# ------------------------------------------------------------- device kernel
_NC_CACHE = {}


def _build_nc():
    if "nc" in _NC_CACHE:
        return _NC_CACHE["nc"]
    f16 = mybir.dt.float16
    f32 = mybir.dt.float32
    nc = bacc.Bacc(None)
    # input laid out [w, img, h]; output [h', img, w'] (host fixes both up)
    u_in = nc.dram_tensor("u", [S, PER_CORE, S], f16, kind="ExternalInput")
    lt_in = nc.dram_tensor("lt", [S, S], f16, kind="ExternalInput")
    r_in = nc.dram_tensor("rm", [S, S], f16, kind="ExternalInput")
    u_out = nc.dram_tensor("out", [S, PER_CORE, S], f16, kind="ExternalOutput")

    with tile.TileContext(nc) as tc:
        with (
            tc.tile_pool(name="mats", bufs=1) as mats,
            tc.tile_pool(name="vt", bufs=16) as vtp,
            tc.tile_pool(name="vrq", bufs=8) as vrp,
            tc.tile_pool(name="outc", bufs=16) as outp,
            tc.tile_pool(name="psA", bufs=5, space="PSUM") as psA,
            tc.tile_pool(name="psB", bufs=3, space="PSUM") as psB,
        ):
            lt_s = mats.tile([S, S], f16)   # L^T
            r_s = mats.tile([S, S], f16)    # R

            # vt pool holds every group, so input DMA issues never block on
            # buffer recycling: Sync finishes issuing ALL inputs early and
            # the output-chunk issues behind them enter the DMA rings as
            # soon as their data is ready. The first groups arrive in
            # 4-image pieces so the PE starts early; weights ride Act's
            # HWDGE queue in parallel.
            vt_tiles = []
            for g in range(NGRP):
                vt = vtp.tile([S, GIMG, S], f16)  # [w, img, h]
                g0 = g * GIMG
                if g < 2:
                    for c in range(0, GIMG, QUAD):
                        nc.sync.dma_start(
                            out=vt[:, c : c + QUAD, :],
                            in_=u_in[:, g0 + c : g0 + c + QUAD, :],
                        )
                        if g == 0 and c == 0:
                            nc.scalar.dma_start(out=r_s[:], in_=r_in[:])
                            nc.scalar.dma_start(out=lt_s[:], in_=lt_in[:])
                else:
                    nc.sync.dma_start(out=vt[:], in_=u_in[:, g0 : g0 + GIMG, :])
                vt_tiles.append(vt)

            # dedicated engines for the two PSUM->SBUF copies
            # (GPSIMD cannot access PSUM on TRN2). mmB is software-pipelined
            # one quad behind mmA: the PE is in-order, so emitting mmB(q)
            # right after copy2(q) would head-of-line-block the next quad's
            # mmA matmuls while copy2 is still on the DVE. With the one-quad
            # lag, copy2 has a full quad of slack before the PE needs it.
            NCH = PER_CORE // OCH
            QPC = OCH // QUAD
            oc_tiles = {}

            def flush_mmB(p):
                vq, c, q = p
                oc = oc_tiles[c]
                pb = psB.tile([S, QUAD, S], f32)  # [h', (4img, w')]
                nc.tensor.matmul(pb[:], lt_s[:], vq[:])
                nc.scalar.copy(oc[:, q * QUAD : (q + 1) * QUAD, :], pb[:])
                if q == QPC - 1:
                    o0 = c * OCH
                    if c == NCH - 1:
                        # split the final chunk so the tail DMA is short
                        h = OCH // 2
                        nc.sync.dma_start(
                            out=u_out[:, o0 : o0 + h, :], in_=oc[:, :h, :]
                        )
                        nc.sync.dma_start(
                            out=u_out[:, o0 + h : o0 + OCH, :], in_=oc[:, h:, :]
                        )
                    else:
                        nc.sync.dma_start(
                            out=u_out[:, o0 : o0 + OCH, :], in_=oc[:]
                        )

            LAG = 2
            pending = []
            for c in range(NCH):
                oc = outp.tile([S, OCH, S], f16, name="oc")  # [h', img, w']
                oc_tiles[c] = oc
                for q in range(QPC):
                    i0 = c * OCH + q * QUAD  # global image index
                    vt = vt_tiles[i0 // GIMG]
                    pa = psA.tile([S, QUAD, S], f32)
                    for j in range(QUAD):
                        nc.tensor.matmul(
                            pa[:, j, :],
                            vt[:, i0 % GIMG + j, :],
                            r_s[:],
                        )
                    vq = vrp.tile([S, QUAD, S], f16)  # [h, (4img, w')]
                    nc.vector.tensor_copy(vq[:], pa[:])
                    pending.append((vq, c, q))
                    if len(pending) > LAG:
                        flush_mmB(pending.pop(0))
            for p in pending:
                flush_mmB(p)

    nc.finalize()
    _NC_CACHE["nc"] = nc
    return nc


def _prep_in_maps(inputs):
    """Host-side prep shared by kernel() and the profiling harness."""
    u = np.asarray(inputs["u"], dtype=np.float32)
    assert u.shape == (BATCH, 1, S, S)
    L, R = _build_lr(
        np.asarray(inputs["alpha_base_x"], dtype=np.float64),
        np.asarray(inputs["alpha_time_coeff_x"], dtype=np.float64),
        np.asarray(inputs["alpha_time_quad_x"], dtype=np.float64),
        np.asarray(inputs["beta_base_y"], dtype=np.float64),
        np.asarray(inputs["beta_time_coeff_y"], dtype=np.float64),
        np.asarray(inputs["beta_time_quad_y"], dtype=np.float64),
    )
    lt16 = np.ascontiguousarray(L.T.astype(np.float16))
    r16 = np.ascontiguousarray(R.astype(np.float16))
    u16 = u[:, 0].astype(np.float16)  # (BATCH, S, S) = [img, h, w]
    return [
        {
            # [w, img, h]: each image uploaded pre-transposed
            "u": np.ascontiguousarray(
                u16[c * PER_CORE : (c + 1) * PER_CORE].transpose(2, 0, 1)
            ),
            "lt": lt16,
            "rm": r16,
        }
        for c in range(N_CORES)
    ]


# ---------------------------------------------------------------- entrypoint
def kernel(**inputs) -> np.ndarray:
    in_maps = _prep_in_maps(inputs)
    nc = _build_nc()
    res = run_bass_kernel_spmd(nc, in_maps, list(range(N_CORES)))
    # device emits [h', img, w'] per core; reassemble to [img, h', w']
    out = np.concatenate([r["out"].transpose(1, 0, 2) for r in res.results], axis=0)
    return np.ascontiguousarray(out).reshape(BATCH, 1, S, S).astype(np.float32)


if __name__ == "__main__":
    rng = np.random.default_rng(0)
    fake = {
        "u": rng.standard_normal((BATCH, 1, S, S), dtype=np.float32),
        "alpha_base_x": np.full(S, 2.0, np.float32),
        "alpha_base_y": np.full(S, 2.0, np.float32),
        "beta_base_x": np.full(S, 2.0, np.float32),
        "beta_base_y": np.full(S, 2.0, np.float32),
        "alpha_time_coeff_x": 0.01 * rng.standard_normal(S).astype(np.float32),
        "alpha_time_coeff_y": 0.01 * rng.standard_normal(S).astype(np.float32),
        "beta_time_coeff_x": 0.01 * rng.standard_normal(S).astype(np.float32),
        "beta_time_coeff_y": 0.01 * rng.standard_normal(S).astype(np.float32),
        "alpha_time_quad_x": 0.01 * rng.standard_normal(S).astype(np.float32),
        "alpha_time_quad_y": 0.01 * rng.standard_normal(S).astype(np.float32),
        "beta_time_quad_x": 0.01 * rng.standard_normal(S).astype(np.float32),
        "beta_time_quad_y": 0.01 * rng.standard_normal(S).astype(np.float32),
    }
    out = kernel(**fake)
    print("kernel output:", out.shape, out.dtype)
